# revision 55
# baseline (speedup 1.0000x reference)
"""Multi-head self-attention Trainium2 kernel (8 NeuronCores).

Problem: x[2,2048,1024] -> qkv proj (w_qkv[1024,3072]) -> 16-head attention
(head_dim 64) -> out proj (w_out[1024,1024]).

Sharding: core c handles batch b=c//4 and head-group g=c%4 (4 heads each).
Each core computes Q/K/V for its 4 heads (tensor-parallel slice of w_qkv),
runs attention for those heads, and computes a partial out-projection
(rows g*256:(g+1)*256 of w_out). The host sums the 4 partials per batch.

On-device layouts (per core):
  XT  [128, 8, 2048]    x^T (d-major), d = dk*128 + p
  QT/KT [128, 2, 2048]  channel-major Q^T/K^T; head h at partitions
                        (h%2)*64..+64 of chunk h//2
  V4  [128, 16, 4, 65]  sequence-major V per k-chunk/head, 65th col = ones
                        (gives the softmax denominator for free in attn@V)
  et  [128, 1024]       exp(scores/8) tiles, k on partitions, q on free
  CTX [128, 2, 2048]    normalized per-head context, channel-major
Matmuls run as float32r (full-rate, fp32 storage, ~1.5e-4 rel err);
PSUM accumulates fp32. Q/K chunk 0 is computed dk-outer against 8 live
PSUM banks so the PE starts as soon as the first x^T chunk lands.
"""

import os
from contextlib import ExitStack

import numpy as np

import concourse.bacc as bacc
import concourse.mybir as mybir
import concourse.tile as tile
from concourse.bass_utils import run_bass_kernel_spmd

P = 128
B, S, D, H, HD = 2, 2048, 1024, 16, 64
HPC = 4          # heads per core
C = HPC * HD     # 256 channels per core
DK = D // P      # 8 contraction chunks
CT = C // P      # 2 channel chunks
SC = S // P      # 16 sequence chunks of 128
NQ = 4           # q chunks of 512
QW = S // NQ     # 512
F32 = mybir.dt.float32
F32R = mybir.dt.float32r
AF = mybir.ActivationFunctionType

N_CORES = 8
CORES_PER_BATCH = 4

# layout of the "cst" constants row: [b_v(256) | 1.0 1.0 | b_out(1024) | ones(128)]
# (two 1.0s so the V-bias matmul has even N — an fp32r ISA requirement)
CST_ONE = C              # index of the 1.0 feeding V4's ones column
CST_BO = C + 2           # b_out
CST_ONES = C + 2 + D     # ones row for K=1 bias/broadcast matmuls
CST_LEN = C + 2 + D + P


def _build():
    nc = bacc.Bacc("TRN2", target_bir_lowering=False, debug=False)
    xt = nc.dram_tensor("xt", (D, S), F32R, kind="ExternalInput")
    wq = nc.dram_tensor("wq", (D, C), F32R, kind="ExternalInput")
    wk = nc.dram_tensor("wk", (D, C), F32R, kind="ExternalInput")
    wv = nc.dram_tensor("wv", (D, C), F32R, kind="ExternalInput")
    wo = nc.dram_tensor("wo", (C, D), F32R, kind="ExternalInput")
    bqk = nc.dram_tensor("bqk", (2, C), F32, kind="ExternalInput")
    cst = nc.dram_tensor("cst", (1, CST_LEN), F32R, kind="ExternalInput")
    out = nc.dram_tensor("out", (D, S), mybir.dt.float16, kind="ExternalOutput")
    boc = nc.dram_tensor("boc", (DK, P), F32, kind="ExternalInput")

    xt_r = xt.rearrange("(dk p) s -> p dk s", p=P)

    with tile.TileContext(nc) as tc, ExitStack() as ctx:
        pers = ctx.enter_context(tc.tile_pool(name="pers", bufs=1))
        QT = pers.tile([P, CT, S], F32R)
        KT = pers.tile([P, CT, S], F32R)
        V4 = pers.tile([P, SC, HPC, HD + 1], F32R)
        CTX = pers.tile([P, CT, S], F32R)
        WO = pers.tile([P, CT, D], F32R)
        BQK = pers.tile([P, 2, CT], F32)   # per-partition bias columns
        BOC = pers.tile([P, DK], F32)      # b_out as per-partition columns
        CST = pers.tile([1, CST_LEN], F32R)

        with tc.tile_pool(name="xw", bufs=1) as xwp:
            XT = xwp.tile([P, DK, S], F32R)
            WQ = xwp.tile([P, DK, C], F32R)
            WK = xwp.tile([P, DK, C], F32R)
            WV = xwp.tile([P, DK, C], F32R)
            # per-chunk loads, interleaved so phase-A dk-step k can start
            # as soon as its (wq, wk, x^T) chunk triplet lands
            wq_r = wq.rearrange("(dk p) c -> p dk c", p=P)
            wk_r = wk.rearrange("(dk p) c -> p dk c", p=P)
            for dk in range(DK - 1):
                if dk == 0:   # x^T chunk first so the first matmul can start
                    nc.sync.dma_start(XT[:, dk, :], xt_r[:, dk, :])
                nc.sync.dma_start(WQ[:, dk, :], wq_r[:, dk, :])
                nc.sync.dma_start(WK[:, dk, :], wk_r[:, dk, :])
                if dk > 0:
                    nc.sync.dma_start(XT[:, dk, :], xt_r[:, dk, :])
            nc.sync.dma_start(WQ[:, DK - 1, :], wq_r[:, DK - 1, :])
            nc.sync.dma_start(WK[:, DK - 1, :], wk_r[:, DK - 1, :])
            nc.sync.dma_start(WV, wv.rearrange("(dk p) c -> p dk c", p=P))
            nc.sync.dma_start(XT[:, DK - 1, :], xt_r[:, DK - 1, :])
            nc.sync.dma_start(BQK, bqk.rearrange("qk (ct p) -> p qk ct", p=P))
            nc.sync.dma_start(BOC, boc.rearrange("nn p -> p nn"))
            nc.sync.dma_start(CST, cst[:, :])
            nc.sync.dma_start(WO, wo.rearrange("(ct p) n -> p ct n", p=P))

            # ---- QKV, one shared 8-bank PSUM pool ----
            # Phase A: Q/K chunk 0 dk-outer over 8 live accumulators, so the
            # PE starts as soon as x^T chunk 0 lands; the final dk step is
            # interleaved with the PSUM->SBUF copies so V can recycle slots.
            with tc.tile_pool(name="psQKV", bufs=8, space="PSUM") as psq:
                acc = []   # (psum, dst, brow, qc)
                for qc in range(NQ):
                    pq = psq.tile([P, QW], F32, tag="qkv", name=f"pq{qc}")
                    pk = psq.tile([P, QW], F32, tag="qkv", name=f"pk{qc}")
                    acc.append((pq, QT, 0, qc))
                    acc.append((pk, KT, 1, qc))
                for dk in range(DK - 1):
                    for pq, dst, brow, qc in acc:
                        wsb = WQ if brow == 0 else WK
                        nc.tensor.matmul(
                            pq, lhsT=wsb[:, dk, 0:P],
                            rhs=XT[:, dk, qc * QW:(qc + 1) * QW],
                            start=(dk == 0), stop=False,
                        )
                for pq, dst, brow, qc in acc:
                    wsb = WQ if brow == 0 else WK
                    nc.tensor.matmul(
                        pq, lhsT=wsb[:, DK - 1, 0:P],
                        rhs=XT[:, DK - 1, qc * QW:(qc + 1) * QW],
                        start=False, stop=True,
                    )
                    nc.scalar.activation(
                        dst[:, 0, qc * QW:(qc + 1) * QW], pq,
                        AF.Identity, bias=BQK[:, brow, 0:1])

                # V (seq-major); slots recycle from phase A as copies finish
                for st in range(SC):
                    ps = psq.tile([P, QW], F32, tag="qkv", name="vps")
                    for dk in range(DK):
                        nc.tensor.matmul(
                            ps[:, :C],
                            lhsT=XT[:, dk, st * P:(st + 1) * P],
                            rhs=WV[:, dk, :],
                            start=(dk == 0), stop=False,
                        )
                    nc.tensor.matmul(
                        ps[:, :C + 2], lhsT=CST[:, CST_ONES:CST_ONES + P],
                        rhs=CST[:, 0:C + 2],
                        start=False, stop=True,
                    )
                    nc.vector.tensor_copy(
                        V4[:, st, :, 0:HD],
                        ps[:, :C].rearrange("p (h d) -> p h d", d=HD))
                    nc.vector.tensor_copy(
                        V4[:, st, :, HD], ps[:, C:C + 1].to_broadcast((P, HPC)))

                # Q/K chunk 1
                for dst, wsb, brow in ((QT, WQ, 0), (KT, WK, 1)):
                    for qc in range(NQ):
                        ps = psq.tile([P, QW], F32, tag="qkv", name="qk1ps")
                        for dk in range(DK):
                            nc.tensor.matmul(
                                ps, lhsT=wsb[:, dk, P:2 * P],
                                rhs=XT[:, dk, qc * QW:(qc + 1) * QW],
                                start=(dk == 0), stop=(dk == DK - 1),
                            )
                        nc.scalar.activation(
                            dst[:, 1, qc * QW:(qc + 1) * QW], ps,
                            AF.Identity, bias=BQK[:, brow, 1:2])

        # ---- attention: 4 passes of (head pair) x (q half) ----
        with (
            tc.tile_pool(name="stp", bufs=2, space="PSUM") as stp,
            tc.tile_pool(name="otp", bufs=4, space="PSUM") as otp,
            tc.tile_pool(name="etp", bufs=4) as etp,
            tc.tile_pool(name="nrm", bufs=4) as nrmp,
        ):
            # 4 passes: (head pair) x (q half). Each St tile packs the two
            # heads of a chunk side by side, so their K=64 score matmuls sit
            # at row groups 0-1 and 2-3 and run concurrently in the PE array.
            for hp in range(2):          # head pair = chunk ct_i
                ct_i = hp
                for qh in range(2):      # q half: chunks 2*qh, 2*qh+1
                    ots = {}
                    for hh in range(2):
                        for j in range(2):
                            ots[hh, j] = otp.tile(
                                [65, QW], F32, tag="ot",
                                name=f"ot{hp}{qh}_{hh}{j}")
                    def attn_v(pend):
                        pet, pkc, pj = pend
                        for hh in range(2):
                            nc.tensor.matmul(
                                ots[hh, pj],
                                lhsT=V4[:, pkc, 2 * hp + hh, :],
                                rhs=pet[:, hh * QW:(hh + 1) * QW],
                                start=(pkc == 0),
                                stop=(pkc == SC - 1),
                            )

                    pends = []    # attn@V trails scores by two tiles: PE
                    for kc in range(SC):   # never blocks on the exp it feeds
                        for j in range(2):
                            qc = qh * 2 + j
                            st = stp.tile([P, 2 * QW], F32, tag="st", name="st")
                            for hh in range(2):
                                nc.tensor.matmul(
                                    st[:, hh * QW:(hh + 1) * QW],
                                    lhsT=KT[hh * 64:(hh + 1) * 64, ct_i,
                                            kc * P:(kc + 1) * P],
                                    rhs=QT[hh * 64:(hh + 1) * 64, ct_i,
                                           qc * QW:(qc + 1) * QW],
                                    start=True,
                                    stop=True,
                                )
                            if len(pends) >= 2:
                                attn_v(pends.pop(0))
                            et = etp.tile([P, 2 * QW], F32R, tag="et", name="et")
                            nc.scalar.activation(et, st, AF.Exp, scale=0.125)
                            pends.append((et, kc, j))
                    for pend in pends:
                        attn_v(pend)
                    # normalize: ctx[c,q] = ot[c,q] * (1/rowsum[q])
                    # j-major so ot banks free in next pass's attn@V order;
                    # all chain ops stay off ACT so next-pass exps aren't
                    # blocked behind them in ACT program order
                    for j in range(2):
                        for hh in range(2):
                            lo, hi = hh * 64, (hh + 1) * 64
                            qc = qh * 2 + j
                            ot = ots[hh, j]
                            rc = nrmp.tile([1, QW], F32R, tag="rc", name="rc")
                            with nc.allow_low_precision(
                                    reason="softmax recip in f32r"):
                                nc.vector.reciprocal(rc, ot[64:65, :])
                            bcps = stp.tile([P, 2 * QW], F32, tag="st",
                                            name="bcps")
                            nc.tensor.matmul(
                                bcps[0:64, 0:QW],
                                lhsT=CST[:, CST_ONES:CST_ONES + 64],
                                rhs=rc,
                                start=True, stop=True,
                            )
                            rcb = nrmp.tile([64, QW], F32, tag="rcb", name="rcb")
                            nc.scalar.copy(rcb, bcps[0:64, 0:QW])
                            nc.vector.tensor_mul(
                                CTX[lo:hi, ct_i, qc * QW:(qc + 1) * QW],
                                ot[0:64, :],
                                rcb,
                            )

        # ---- out projection, transposed: out^T[n, s] ----
        # out^T = W_out^T-chunk @ CTX; b_out lands on partitions, so the
        # bias rides the PSUM->SBUF copy and no bias matmuls are needed
        with (
            tc.tile_pool(name="ops", bufs=4, space="PSUM") as opsp,
            tc.tile_pool(name="osb", bufs=4) as osbp,
        ):
            for nn in range(DK):
                osb = osbp.tile([P, S], mybir.dt.float16, tag="osb", name="osb")
                for sq in range(NQ):
                    ps = opsp.tile([P, QW], F32, tag="o", name="ops")
                    for cc in range(CT):
                        nc.tensor.matmul(
                            ps,
                            lhsT=WO[:, cc, nn * P:(nn + 1) * P],
                            rhs=CTX[:, cc, sq * QW:(sq + 1) * QW],
                            start=(cc == 0),
                            stop=(cc == CT - 1),
                        )
                    if sq % 2 == 0:
                        nc.vector.tensor_scalar_add(
                            osb[:, sq * QW:(sq + 1) * QW], ps, BOC[:, nn:nn + 1])
                    else:
                        nc.scalar.activation(
                            osb[:, sq * QW:(sq + 1) * QW], ps, AF.Identity,
                            bias=BOC[:, nn:nn + 1])
                nc.sync.dma_start(out[nn * P:(nn + 1) * P, :], osb)

    nc.compile()
    return nc


_NC = None


def kernel(x, w_qkv, b_qkv, w_out, b_out):
    global _NC
    x = np.asarray(x, dtype=np.float32)
    w_qkv = np.asarray(w_qkv, dtype=np.float32)
    b_qkv = np.asarray(b_qkv, dtype=np.float32)
    w_out = np.asarray(w_out, dtype=np.float32)
    b_out = np.asarray(b_out, dtype=np.float32)

    if _NC is None:
        _NC = _build()

    in_maps = []
    for core in range(N_CORES):
        b_i, g = divmod(core, CORES_PER_BATCH)
        h0 = g * HPC
        cs = slice(h0 * HD, (h0 + HPC) * HD)          # this core's channels
        qs, ks, vs = (np.ascontiguousarray(w_qkv[:, i * D:(i + 1) * D][:, cs])
                      for i in range(3))
        bo_eff = b_out if g == 0 else np.zeros_like(b_out)
        cst_row = np.concatenate(
            [b_qkv[2 * D:3 * D][cs], [1.0, 1.0], bo_eff, np.ones(P, np.float32)]
        ).astype(np.float32)[None, :]
        in_maps.append({
            "xt": np.ascontiguousarray(x[b_i].T),
            "wq": qs,
            "wk": ks,
            "wv": vs,
            "wo": np.ascontiguousarray(w_out[cs, :]),
            "bqk": np.ascontiguousarray(
                np.stack([b_qkv[0 * D:1 * D][cs], b_qkv[1 * D:2 * D][cs]])),
            "boc": np.ascontiguousarray(bo_eff.reshape(DK, P)),
            "cst": cst_row,
        })

    trace = bool(int(os.environ.get("BASS_KERNEL_TRACE", "0")))
    res = run_bass_kernel_spmd(
        _NC, in_maps, core_ids=list(range(N_CORES)), trace=trace,
    )
    if trace and res.exec_time_ns is not None:
        print(f"HW exec time: {res.exec_time_ns} ns")
        if res.instructions_and_trace is not None:
            print(f"trace: {res.instructions_and_trace[1]}")

    outs = [r["out"] for r in res.results]
    full = np.empty((B, S, D), dtype=np.float32)
    for b_i in range(B):
        full[b_i] = np.sum(
            np.stack(outs[b_i * CORES_PER_BATCH:(b_i + 1) * CORES_PER_BATCH]),
            axis=0, dtype=np.float32,
        ).T
    return full
